# revision 1
# baseline (speedup 1.0000x reference)
"""De-stationary causal attention (B=2, L=S=2048, H=8, E=64) on 8 TRN2 cores.

Sharding: the 16 (batch, head) pairs are distributed 2-per-core (cores 0-3
get batch 0, heads 0..7; cores 4-7 get batch 1). Each core runs the same
Bass program on its two pairs.

Math: logits = (Q K^T) * (tau/sqrt(E)) + delta/sqrt(E), causal softmax, A V.
Host-side folds: Q is pre-scaled by tau/sqrt(E); exp(delta/sqrt(E)) is folded
into V (and into the appended denominator column), because
softmax(x + d)_s = exp(x_s) e^{d_s} / sum_j exp(x_j) e^{d_j}.
So the device only computes exp(q'k) with no bias, letting one ACT call span
a whole 4-bank PSUM group. The device returns the UNNORMALIZED accumulator
OT[e, l] (rows 0..63 = sum_s A_sl V_se, row 64 = denominator); the host does
the final divide + [e,l] -> [l,e] transpose during unshard.

Device structure per (b,h) pair, scores kept TRANSPOSED (s on partitions):
  bank-major over 4 output l-blocks of 512; for each bank, groups of 4
  s-tiles: ST[s,l] row-packed on the PE (two k=64 matmuls on partition halves
  run concurrently), one exp over each [128,1024] half-group, causal mask on
  diag blocks (DVE), then AV row-packed as well: the s=128 contraction is
  split 64+64 into two accumulators otA/otB (row groups h0/h64 run
  concurrently), merged by one DVE add per bank straight to SBUF and DMA'd
  out. Q^T/K^T are duplicated onto both partition halves ON HOST so each
  arrives in a single DMA per 512-column chunk, spread across the sync /
  scalar / gpsimd queues to minimize time-to-first-matmul.
"""

import copy
import sys

import numpy as np

try:
    import concourse.bass as bass
except ImportError:  # pragma: no cover
    sys.path.insert(0, "/opt/trn_rl_repo")
    import concourse.bass as bass

import concourse.mybir as mybir
import concourse.tile as tile
from concourse.bass_utils import run_bass_kernel_spmd
from concourse.vector_clock import ScopedClock

B, L, H, E = 2, 2048, 8, 64
N_CORES = 8
PAIRS_PER_CORE = 2
SCALE = 1.0 / np.sqrt(np.float32(E))  # 0.125

f32 = mybir.dt.float32
f32r = mybir.dt.float32r
bf16 = mybir.dt.bfloat16

# ---------------------------------------------------------------------------
# Walrus in this toolchain rejects >1 sync-wait per instruction. Split extra
# waits onto NoOps committed just before the instruction on the same engine.
# ---------------------------------------------------------------------------
_NOP_TEMPLATE = {}


def _make_nop(engine, name):
    if engine not in _NOP_TEMPLATE:
        tmp = bass.Bass()
        _NOP_TEMPLATE[engine] = tmp.engines[engine].nop(nofuse=True).ins
    nop = copy.copy(_NOP_TEMPLATE[engine])
    nop.name = name
    nop.engine = engine
    nop.sync_info = None
    return nop


class SplitWaitTileContext(tile.TileContext):
    _ws_counter = 0

    def _split_waits(self, inst):
        si = inst.sync_info
        if si is None or not si.on_wait or len(si.on_wait) <= 1:
            return []
        if inst.engine == mybir.EngineType.Unassigned:
            return []
        waits = list(si.on_wait)
        inst.sync_info = mybir.SyncInfo(
            on_wait=[waits[0]], on_update=list(si.on_update or [])
        )
        nops = []
        for w in waits[1:]:
            SplitWaitTileContext._ws_counter += 1
            nop = _make_nop(inst.engine, f"I-ws{SplitWaitTileContext._ws_counter}")
            nop.sync_info = mybir.SyncInfo(on_wait=[w], on_update=[])
            nops.append(nop)
        return nops

    def _commit_instruction(self, inst, lazy_reg_writes=True):
        for nop in self._split_waits(inst):
            self._add_instruction(nop)
        super()._commit_instruction(inst, lazy_reg_writes)

    def _drain_and_barrier(self, tick_clock, wait_clock):
        nc = self.nc
        probe = nc.sync.nop(nofuse=True)
        wait_clock.add_sem_waits(
            probe.ins, ScopedClock({None: tick_clock.global_clock})
        )
        waits = list(probe.ins.sync_info.on_wait or []) if probe.ins.sync_info else []
        if len(waits) > 1:
            probe.ins.sync_info.on_wait = [waits[0]]
            handles = {h.num: h for h in self.sems.allocated().values()}
            for w in waits[1:]:
                nop = nc.sync.nop(nofuse=True)
                nop.wait_op(handles[w.id], w.wait_value, "sem-ge")
        nc.sync.drain()

        nc.all_engine_barrier()
        assert self.sems is not None
        popped = nc._tile_sem_poison_stack.pop()
        assert popped is self._sem_poison
        nc.clear_and_free_semaphores(list(self.sems.allocated().values()))


# ---------------------------------------------------------------------------
# Program builder (bank-major, fully row-packed, host-normalized epilogue)
# ---------------------------------------------------------------------------

def build_program(st_dtype=bf16, av_dtype=bf16):
    nc = bass.Bass()
    Exp = mybir.ActivationFunctionType.Exp

    VW = E + 2  # v row: 64 values + denominator col + pad
    # qt/kt are duplicated on both partition halves HOST-SIDE -> [128, L]
    qt = nc.declare_dram_parameter("qt", [PAIRS_PER_CORE, 128, L], st_dtype, isOutput=False)
    kt = nc.declare_dram_parameter("kt", [PAIRS_PER_CORE, 128, L], st_dtype, isOutput=False)
    vv = nc.declare_dram_parameter("vv", [PAIRS_PER_CORE, L, VW], av_dtype, isOutput=False)
    mask = nc.declare_dram_parameter("mask", [128, 128], av_dtype, isOutput=False)
    # unnormalized output: rows 0..63 = (A V)^T, row 64 = softmax denominator
    oo = nc.declare_dram_parameter("oo", [PAIRS_PER_CORE, E + 1, L], f32, isOutput=True)

    NT = L // 128  # 16 s-tiles / l-tiles
    NB = L // 512  # 4 output banks

    with SplitWaitTileContext(nc) as tc:
        with (
            tc.tile_pool(name="const", bufs=1) as constp,
            tc.tile_pool(name="qk", bufs=2) as qkp,
            tc.tile_pool(name="vp", bufs=2) as vp,
            tc.tile_pool(name="ap", bufs=4) as ap_pool,
            tc.tile_pool(name="ep", bufs=2) as ep,
            tc.tile_pool(name="st", bufs=1, space="PSUM") as stp,
            tc.tile_pool(name="ot", bufs=2, space="PSUM") as otp,
        ):
            mask_sb = constp.tile([128, 128], av_dtype, tag="mask")

            # -- tiles for both pairs (bufs=2 pools keep both resident) -----
            tiles = []
            for pair in range(PAIRS_PER_CORE):
                qt_sb = qkp.tile([128, L], st_dtype, tag="qt")
                kt_sb = qkp.tile([128, L], st_dtype, tag="kt")
                v_sb = vp.tile([128, NT, VW], av_dtype, tag="v")
                tiles.append((qt_sb, kt_sb, v_sb))

            # -- input loads, spread across the three DMA-capable queues ----
            # (sync + scalar are HWDGE; gpsimd is SWDGE). Priority order is
            # first-use order of the group schedule; scalar only takes the
            # two earliest kt chunks so the ACT stream is never delayed.
            def chunk_loads(pair, qdst, kdst, vdst):
                vv_r = vv[pair].rearrange("(t p) e -> p t e", p=128)
                q = lambda ch: (qdst[:, 512 * ch : 512 * (ch + 1)],
                                qt[pair][:, 512 * ch : 512 * (ch + 1)])
                k = lambda ch: (kdst[:, 512 * ch : 512 * (ch + 1)],
                                kt[pair][:, 512 * ch : 512 * (ch + 1)])
                v = lambda ch: (vdst[:, 4 * ch : 4 * ch + 4, :],
                                vv_r[:, 4 * ch : 4 * ch + 4, :])
                return q, k, v

            # bank order [1, 2, 3, 0]: the final bank has a single AV group,
            # shortening the post-last-ACT tail chain. First-use order of
            # chunks follows that schedule: qt1,kt0 / kt1 / qt2 / kt2 ...
            q0, k0, v0 = chunk_loads(0, tiles[0][0], tiles[0][1], tiles[0][2])
            q1, k1, v1 = chunk_loads(1, tiles[1][0], tiles[1][1], tiles[1][2])
            nc.gpsimd.dma_start(out=mask_sb, in_=mask[:])
            for eng, (dst, src) in [
                (nc.sync, q0(1)), (nc.scalar, k0(0)),
                (nc.sync, q0(2)), (nc.scalar, k0(1)),
                (nc.sync, v0(0)), (nc.gpsimd, k0(2)),
                (nc.sync, q0(3)), (nc.gpsimd, k0(3)),
                (nc.sync, q0(0)), (nc.gpsimd, v0(1)),
                (nc.gpsimd, v0(2)), (nc.gpsimd, v0(3)),
                (nc.sync, q1(1)), (nc.gpsimd, k1(0)),
                (nc.sync, q1(2)), (nc.gpsimd, k1(1)),
                (nc.sync, v1(0)), (nc.sync, q1(3)),
                (nc.gpsimd, k1(2)), (nc.sync, q1(0)),
                (nc.gpsimd, k1(3)), (nc.gpsimd, v1(1)),
                (nc.gpsimd, v1(2)), (nc.gpsimd, v1(3)),
            ]:
                eng.dma_start(out=dst, in_=src)

            # -- compute ---------------------------------------------------
            # single software-pipelined schedule across BOTH pairs: the PE
            # stays one ST group ahead of AV, and pair 1's first ST group
            # hides pair 0's final AV + merge (no ACT gap at the boundary)
            if True:
                ot_banks = {}

                def emit_st_group(pair, lj, gi):
                    qt_sb, kt_sb, v_sb = tiles[pair]
                    a_grp = ap_pool.tile(
                        [128, 4 * 512], av_dtype, tag="A", name="A"
                    )
                    diag = gi == lj
                    for hb in range(2):  # two ping-ponged [128,1024] halves
                        st = stp.tile(
                            [128, 1024], f32, tag=f"st{hb}", name="st"
                        )
                        for cc in range(2):
                            w = 2 * hb + cc
                            # diagonal groups place chunk c in window 3-c so
                            # the valid suffixes of windows 2,3 coalesce into
                            # one contiguous ACT region (4 -> 3 ACTs/group)
                            c = 3 - w if diag else w
                            si = 4 * gi + c
                            off = 128 * c if diag else 0
                            half = (w % 2) * E
                            nc.tensor.matmul(
                                st[:, 512 * cc + off : 512 * (cc + 1)],
                                kt_sb[half : half + E, si * 128 : si * 128 + 128],
                                qt_sb[half : half + E, 512 * lj + off : 512 * lj + 512],
                                start=True,
                                stop=True,
                            )
                        if diag:
                            if hb == 0:
                                # windows 0,1 hold chunks 3,2: valid suffixes
                                # st[384:512] and st[768:1024]
                                for cc in range(2):
                                    off = 128 * (3 - cc)
                                    nc.scalar.activation(
                                        out=a_grp[:, 512 * cc + off : 512 * (cc + 1)],
                                        in_=st[:, 512 * cc + off : 512 * (cc + 1)],
                                        func=Exp,
                                        scale=1.0,
                                    )
                            else:
                                # windows 2,3 hold chunks 1,0: suffixes
                                # [128,512)+[512,1024) merge to st[128:1024]
                                nc.scalar.activation(
                                    out=a_grp[:, 1024 + 128 : 2048],
                                    in_=st[:, 128:1024],
                                    func=Exp,
                                    scale=1.0,
                                )
                        else:
                            nc.scalar.activation(
                                out=a_grp[:, 1024 * hb : 1024 * (hb + 1)],
                                in_=st,
                                func=Exp,
                                scale=1.0,
                            )
                    if diag:
                        for c in range(4):
                            colb = 512 * (3 - c) + 128 * c
                            nc.vector.tensor_mul(
                                a_grp[:, colb : colb + 128],
                                a_grp[:, colb : colb + 128],
                                mask_sb,
                            )
                    return a_grp

                def emit_av_group(pair, lj, gi, a_grp):
                    # k=128 contraction split 64+64 -> two accumulators on
                    # PE row groups h0/h64 (concurrent streams)
                    v_sb = tiles[pair][2]
                    otA, otB = ot_banks[(pair, lj)]
                    diag = gi == lj
                    for c in range(4):
                        si = 4 * gi + c
                        w = 3 - c if diag else c
                        off = 128 * c if diag else 0
                        for half, ot in ((0, otA), (1, otB)):
                            p0 = 64 * half
                            nc.tensor.matmul(
                                ot[:, off:512],
                                v_sb[p0 : p0 + 64, si, 0 : E + 1],
                                a_grp[p0 : p0 + 64, 512 * w + off : 512 * w + 512],
                                start=(gi == 0 and c == 0),
                                stop=(diag and c == 3),
                            )

                def emit_merge(pair, lj):
                    # DVE can read only one PSUM operand per instruction:
                    # evacuate otA first, then add otB into the SBUF copy.
                    otA, otB = ot_banks.pop((pair, lj))
                    ot_sb = ep.tile([E + 1, 512], f32, tag="osb", name="osb")
                    nc.vector.tensor_copy(ot_sb, otA)
                    nc.vector.tensor_add(ot_sb, ot_sb, otB)
                    nc.sync.dma_start(
                        out=oo[pair][:, 512 * lj : 512 * (lj + 1)], in_=ot_sb
                    )

                # groups: (pair, lj, gi) — bank lj accumulates s-tiles
                # 0..4lj+3 in groups of 4; gi == lj is the diagonal (partial)
                # group. Banks in order [1,2,3,0] so the last bank has a
                # single AV group (short tail).
                groups = [
                    (pair, lj, gi)
                    for pair in range(PAIRS_PER_CORE)
                    for lj in (1, 2, 3, 0)
                    for gi in range(lj + 1)
                ]
                # PE p-state warm-up: the Tensor clock needs ~3us of
                # continuous execution to reach 2.4GHz. Run junk matmuls on
                # a zeroed SBUF tile while the first input DMAs are still in
                # flight, so the first real groups run at full clock.
                warm_sb = constp.tile([128, 640], st_dtype, tag="warm")
                nc.vector.memset(warm_sb, 0)
                warm_st = stp.tile([128, 1024], f32, tag="st0", name="warm")
                for _ in range(5):
                    for half in range(2):
                        p0 = 64 * half
                        nc.tensor.matmul(
                            warm_st[:, 512 * half : 512 * (half + 1)],
                            warm_sb[p0 : p0 + 64, 0:128],
                            warm_sb[p0 : p0 + 64, 128:640],
                            start=True,
                            stop=True,
                        )

                prev = None
                for pair, lj, gi in groups:
                    if (pair, lj) not in ot_banks:
                        ot_banks[(pair, lj)] = (
                            otp.tile([E + 1, 512], f32, tag="otA", name="otA"),
                            otp.tile([E + 1, 512], f32, tag="otB", name="otB"),
                        )
                    a_grp = emit_st_group(pair, lj, gi)
                    if prev is not None:
                        pp, plj, pgi, pa = prev
                        emit_av_group(pp, plj, pgi, pa)
                        if pgi == plj:  # last group of bank (pp, plj)
                            emit_merge(pp, plj)
                    prev = (pair, lj, gi, a_grp)
                # final bank (single diagonal group): cols [0,384) of the
                # accumulators are final after chunk 2 — chunk 3 only writes
                # [384,512). Merge + DMA them while chunk 3 still runs, so
                # the post-last-matmul tail is only the 128-col remainder.
                pp, plj, pgi, pa = prev
                fv_sb = tiles[pp][2]
                otA, otB = ot_banks.pop((pp, plj))
                ot_sb = ep.tile([E + 1, 512], f32, tag="osb", name="osb")
                base = 512 * plj
                for c in range(4):
                    if c == 3:
                        nc.vector.tensor_copy(ot_sb[:, 0:384], otA[:, 0:384])
                        nc.vector.tensor_add(
                            ot_sb[:, 0:384], ot_sb[:, 0:384], otB[:, 0:384]
                        )
                        nc.sync.dma_start(
                            out=oo[pp][:, base : base + 384], in_=ot_sb[:, 0:384]
                        )
                    si = 4 * pgi + c
                    w = 3 - c
                    off = 128 * c
                    for half, ot in ((0, otA), (1, otB)):
                        p0 = 64 * half
                        nc.tensor.matmul(
                            ot[:, off:512],
                            fv_sb[p0 : p0 + 64, si, 0 : E + 1],
                            pa[p0 : p0 + 64, 512 * w + off : 512 * w + 512],
                            start=(c == 0),
                            stop=(c == 3),
                            skip_group_check=True,
                        )
                nc.vector.tensor_copy(ot_sb[:, 384:512], otA[:, 384:512])
                nc.vector.tensor_add(
                    ot_sb[:, 384:512], ot_sb[:, 384:512], otB[:, 384:512]
                )
                nc.scalar.dma_start(
                    out=oo[pp][:, base + 384 : base + 512], in_=ot_sb[:, 384:512]
                )

    return nc


# ---------------------------------------------------------------------------
# Host-side sharding / unsharding
# ---------------------------------------------------------------------------

def _in_maps(queries, keys, values, tau, delta, st_dtype=bf16, av_dtype=bf16):
    np_st = mybir.dt.np(st_dtype)
    np_av = mybir.dt.np(av_dtype)
    mask = np.triu(np.ones((128, 128), dtype=np.float32)).astype(np_av)
    maps = []
    for c in range(N_CORES):
        ps = [2 * c, 2 * c + 1]
        b = ps[0] // H
        hs = [p % H for p in ps]
        qscale = np.float32(SCALE * tau[b, 0])
        # q/k transposed [E, L] and duplicated onto both partition halves
        qt = np.ascontiguousarray(
            np.stack([
                np.concatenate([queries[b, :, h, :].T * qscale] * 2, axis=0)
                for h in hs
            ])
        ).astype(np_st)
        kt = np.ascontiguousarray(
            np.stack([
                np.concatenate([keys[b, :, h, :].T] * 2, axis=0) for h in hs
            ])
        ).astype(np_st)
        # V augmented with the delta fold: cols 0..63 = V * exp(delta'),
        # col 64 = exp(delta') (denominator), col 65 pad
        expd = np.exp(SCALE * delta[b]).astype(np.float32)  # [L]
        vv = np.zeros((PAIRS_PER_CORE, L, E + 2), dtype=np.float32)
        for i, h in enumerate(hs):
            vv[i, :, 0:E] = values[b, :, h, :] * expd[:, None]
            vv[i, :, E] = expd
        vv = np.ascontiguousarray(vv).astype(np_av)
        maps.append({"qt": qt, "kt": kt, "vv": vv, "mask": mask})
    return maps


_CACHED = {}


def run(queries, keys, values, tau, delta, trace=False, st_dtype=bf16,
        av_dtype=bf16):
    key = (str(st_dtype), str(av_dtype))
    if key not in _CACHED:
        _CACHED[key] = build_program(st_dtype, av_dtype)
    nc = _CACHED[key]
    in_maps = _in_maps(
        np.asarray(queries),
        np.asarray(keys),
        np.asarray(values),
        np.asarray(tau),
        np.asarray(delta),
        st_dtype=st_dtype,
        av_dtype=av_dtype,
    )
    res = run_bass_kernel_spmd(
        nc, in_maps, core_ids=list(range(N_CORES)), trace=trace
    )
    out = np.empty((B, L, H, E), dtype=np.float32)
    for c in range(N_CORES):
        o = np.asarray(res.results[c]["oo"], dtype=np.float32)  # [2, 65, L]
        for i, p in enumerate([2 * c, 2 * c + 1]):
            out[p // H, :, p % H, :] = (o[i, 0:E, :] / o[i, E : E + 1, :]).T
    return out, res


def kernel(queries, keys, values, tau, delta):
    out, _ = run(queries, keys, values, tau, delta, trace=False)
    return out

